# revision 21
# baseline (speedup 1.0000x reference)
import sys

sys.path.insert(0, "/opt/trn_rl_repo")

import numpy as np

D_MODEL = 1024
NUM_HEADS = 16
HEAD_DIM = 64
B = 2
S = 2048
N_CORES = 8
HG = 4          # head-groups (cores per batch)
HPC = 4         # heads per core
DL = 256        # local feature width per core (HPC * HEAD_DIM)

_cache = {}
last_exec_time_ns = None

DRIP = 4        # filler instructions interleaved per attention m-step


def _build(has_qkvb):
    import concourse.bacc as bacc
    import concourse.mybir as mybir
    import concourse.tile as tile

    F32 = mybir.dt.float32
    F32R = mybir.dt.float32r
    BF16 = mybir.dt.bfloat16
    Exp = mybir.ActivationFunctionType.Exp
    mult = mybir.AluOpType.mult
    is_ge = mybir.AluOpType.is_ge

    nc = bacc.Bacc("TRN2", target_bir_lowering=False, debug=False)
    # packed layouts: [128 partitions, tile index, cols]
    xT_d = nc.dram_tensor("xTh", (128, 8, S), BF16, kind="ExternalInput")
    wq_d = nc.dram_tensor("wqh", (128, 8, 3 * DL), BF16, kind="ExternalInput")
    wo_d = nc.dram_tensor("woh", (128, 2, D_MODEL), BF16, kind="ExternalInput")
    if has_qkvb:
        qb_d = nc.dram_tensor("qb", (1, 3 * DL), BF16, kind="ExternalInput")
    out_d = nc.dram_tensor("out", (S, D_MODEL), BF16, kind="ExternalOutput")

    with tile.TileContext(nc) as tc:
        with tc.tile_pool(name="persist", bufs=1) as persist, \
             tc.tile_pool(name="work", bufs=1) as work, \
             tc.tile_pool(name="pmm", bufs=1, space="PSUM") as pmm:

            xtb = persist.tile([128, 8, S], BF16, name="xtb")
            wqb = persist.tile([128, 8, 3 * DL], BF16, name="wqb")
            wob = persist.tile([128, 2, D_MODEL], BF16, name="wob")
            # Q/K packed per head-pair p: partitions 0:64 head 2p, 64:128 head 2p+1
            QT = [persist.tile([128, S], BF16, name=f"QT{p}") for p in range(2)]
            KT = [persist.tile([128, S], BF16, name=f"KT{p}") for p in range(2)]
            # V augmented [pair, parity, S]: per key-tile block of 128 cols:
            # [V dims 64 | ones 64]; partitions of block st = keys of tile st
            Vaug = persist.tile([128, 2, 2, S], BF16, name="Vaug")
            ctxp = [persist.tile([128, S], BF16, name=f"ctxp{p}") for p in range(2)]
            maskf = persist.tile([128, 128], F32, name="maskf")
            maskb = persist.tile([128, 128], BF16, name="maskb")

            # DMAs in need order. wq columns are host-reordered as
            # [Q-pair0 | K-pair0 | V | Q-pair1 | K-pair1] so the critical
            # pair-0 QK weights land first. The first chunk is split per
            # i-tile so the leading projection matmuls start as soon as
            # their own slice arrives.
            nc.sync.dma_start(out=wqb[:, :, 0:256], in_=wq_d[:, :, 0:256])
            for i in range(8):
                nc.sync.dma_start(out=xtb[:, i, 0:512], in_=xT_d[:, i, 0:512])
            nc.sync.dma_start(out=wqb[:, :, 256:512], in_=wq_d[:, :, 256:512])
            if has_qkvb:
                qb_t = persist.tile([1, 3 * DL], BF16, name="qb_t")
                nc.sync.dma_start(out=qb_t[:], in_=qb_d[:])
                ones_t = persist.tile([1, 512], BF16, name="ones_t")
                nc.vector.memset(ones_t[:], 1.0)
            nc.sync.dma_start(out=wob[:], in_=wo_d[:])
            nc.sync.dma_start(out=xtb[:, :, 512:1024], in_=xT_d[:, :, 512:1024])
            nc.sync.dma_start(out=wqb[:, :, 512:768], in_=wq_d[:, :, 512:768])
            nc.sync.dma_start(out=xtb[:, :, 1024:1536], in_=xT_d[:, :, 1024:1536])
            nc.sync.dma_start(out=xtb[:, :, 1536:2048], in_=xT_d[:, :, 1536:2048])

            # causal diag-block mask: maskb[k, q] = 1 if q >= k (within 128x128 tile)
            nc.vector.memset(maskf[:], 1.0)
            nc.gpsimd.affine_select(
                out=maskf[:].bitcast(F32R), in_=maskf[:].bitcast(F32R),
                pattern=[[1, 128]],
                channel_multiplier=-1,
                base=0,
                compare_op=is_ge,
                fill=0.0,
            )
            nc.vector.tensor_copy(out=maskb[:], in_=maskf[:])
            nc.vector.memset(Vaug[:], 1.0)

            # ---- projection / out-projection instruction generators ----
            MIOFF = {0: 0, 2: 128, 1: 512, 3: 640}

            def qk_items(mi, n):
                # psq = sum_i wq[i][:, mi-block].T @ xt[i][:, n-chunk]  -> [128 feat, 512 seq]
                items = []
                st = {}

                def mk(i):
                    def f():
                        if i == 0:
                            st['ps'] = pmm.tile([128, 512], F32, tag="pp", bufs=2, name="psq")
                        nc.tensor.matmul(
                            out=st['ps'][:],
                            lhsT=wqb[:, i, MIOFF[mi]:MIOFF[mi] + 128],
                            rhs=xtb[:, i, 512 * n:512 * (n + 1)],
                            start=(i == 0),
                            stop=(i == 7 and not has_qkvb),
                        )
                    return f
                for i in range(8):
                    items.append(mk(i))
                if has_qkvb:
                    def fb():
                        nc.tensor.matmul(
                            out=st['ps'][:],
                            lhsT=qb_t[0:1, MIOFF[mi]:MIOFF[mi] + 128],
                            rhs=ones_t[0:1, :],
                            start=False, stop=True,
                        )
                    items.append(fb)

                def cp():
                    dst = QT[mi] if mi < 2 else KT[mi - 2]
                    nc.vector.tensor_copy(
                        out=dst[:, 512 * n:512 * (n + 1)], in_=st['ps'][:])
                items.append(cp)
                return items

            def v_items(sti):
                # psv = sum_i xt[i][:, st-block].T @ wq[i][:, V cols] -> [128 seq, 256 feat]
                items = []
                st = {}

                def mk(i):
                    def f():
                        if i == 0:
                            st['ps'] = pmm.tile([128, 2, 128], F32, tag="pp", bufs=2, name="psv")
                        nc.tensor.matmul(
                            out=st['ps'][:],
                            lhsT=xtb[:, i, 128 * sti:128 * (sti + 1)],
                            rhs=wqb[:, i, 256:512],
                            start=(i == 0),
                            stop=(i == 7 and not has_qkvb),
                        )
                    return f
                for i in range(8):
                    items.append(mk(i))
                if has_qkvb:
                    def fb():
                        nc.tensor.matmul(
                            out=st['ps'][:],
                            lhsT=ones_t[0:1, 0:128],
                            rhs=qb_t[0:1, 256:512],
                            start=False, stop=True,
                        )
                    items.append(fb)

                def cpe():
                    nc.vector.tensor_copy(
                        out=Vaug[:, :, 0, 128 * sti:128 * sti + 64],
                        in_=st['ps'][:, :, 0:64])

                def cpo():
                    nc.vector.tensor_copy(
                        out=Vaug[:, :, 1, 128 * sti:128 * sti + 64],
                        in_=st['ps'][:, :, 64:128])
                items.append(cpe)
                items.append(cpo)
                return items

            def outproj_items(qm):
                items = []
                st = {}

                def half(n):
                    def f():
                        ps = pmm.tile([128, 512], F32, tag="pp", bufs=2, name="pso")
                        nc.tensor.matmul(
                            out=ps[:],
                            lhsT=ctxp[0][:, 128 * qm:128 * (qm + 1)],
                            rhs=wob[:, 0, 512 * n:512 * (n + 1)],
                            start=True, stop=False,
                        )
                        nc.tensor.matmul(
                            out=ps[:],
                            lhsT=ctxp[1][:, 128 * qm:128 * (qm + 1)],
                            rhs=wob[:, 1, 512 * n:512 * (n + 1)],
                            start=False, stop=True,
                        )
                        if n == 0:
                            st['stage'] = work.tile([128, D_MODEL], BF16, tag="st", bufs=3, name="stage")
                        nc.vector.tensor_copy(
                            out=st['stage'][:, 512 * n:512 * (n + 1)], in_=ps[:])
                    return f
                items.append(half(0))
                items.append(half(1))

                def dm():
                    nc.sync.dma_start(out=out_d[128 * qm:128 * (qm + 1), :], in_=st['stage'][:])
                items.append(dm)
                return items

            # ---- attention scores issue ----
            def issue_scores(p, j, m):
                t = m - 4 * j
                lo = 128 * t if t > 0 else 0
                ps = pmm.tile([128, 2, 512], F32, tag="s", bufs=2, name="psS")
                nc.tensor.matmul(
                    out=ps[:, 0, lo:512],
                    lhsT=KT[p][0:64, 128 * m:128 * (m + 1)],
                    rhs=QT[p][0:64, 512 * j + lo:512 * (j + 1)],
                    start=True, stop=True,
                )
                nc.tensor.matmul(
                    out=ps[:, 1, lo:512],
                    lhsT=KT[p][64:128, 128 * m:128 * (m + 1)],
                    rhs=QT[p][64:128, 512 * j + lo:512 * (j + 1)],
                    start=True, stop=True,
                )
                return ps

            # ---- immediate emission: minimum needed for attn(0, 0) ----
            # qk(0,0) and qk(2,0) interleaved per i so each matmul runs as its
            # x/w slice lands
            psq0 = pmm.tile([128, 512], F32, tag="pp", bufs=2, name="psq0")
            psq2 = pmm.tile([128, 512], F32, tag="pp", bufs=2, name="psq2")
            for i in range(8):
                for mi, pst in ((0, psq0), (2, psq2)):
                    nc.tensor.matmul(
                        out=pst[:],
                        lhsT=wqb[:, i, MIOFF[mi]:MIOFF[mi] + 128],
                        rhs=xtb[:, i, 0:512],
                        start=(i == 0),
                        stop=(i == 7 and not has_qkvb),
                    )
            if has_qkvb:
                for mi, pst in ((0, psq0), (2, psq2)):
                    nc.tensor.matmul(
                        out=pst[:],
                        lhsT=qb_t[0:1, MIOFF[mi]:MIOFF[mi] + 128],
                        rhs=ones_t[0:1, :],
                        start=False, stop=True,
                    )
            nc.vector.tensor_copy(out=QT[0][:, 0:512], in_=psq0[:])
            nc.vector.tensor_copy(out=KT[0][:, 0:512], in_=psq2[:])
            psprev = issue_scores(0, 0, 0)
            for sti in range(4):
                for it in v_items(sti):
                    it()

            # ---- filler queue for the rest, drained during attention ----
            FQ = []
            need_idx = {(0, 0): 0}
            for n in range(1, 4):
                FQ += qk_items(0, n) + qk_items(2, n)
                for sti in range(4 * n, 4 * n + 4):
                    FQ += v_items(sti)
                need_idx[(0, n)] = len(FQ)
            for n in range(4):
                FQ += qk_items(1, n) + qk_items(3, n)
                need_idx[(1, n)] = len(FQ)

            drained = [0]

            def drain_to(k):
                while drained[0] < k:
                    FQ[drained[0]]()
                    drained[0] += 1

            def drip(r):
                drain_to(min(drained[0] + r, len(FQ)))

            # pair-1 blocks in descending j so the final out-proj chunk is small
            blocks = [(0, 0), (0, 1), (0, 2), (0, 3), (1, 3), (1, 2), (1, 1), (1, 0)]
            # late blocks drip slower so out-proj filler spreads across the
            # starved final phase instead of draining in a burst
            drip_rate = {(1, 3): 4, (1, 2): 2, (1, 1): 2, (1, 0): 3}
            for bi, (p, j) in enumerate(blocks):
                drain_to(need_idx[(p, j)])
                rate = drip_rate.get((p, j), DRIP)
                mlast = 4 * j + 3
                psA = pmm.tile([128, 512], F32, tag="a", bufs=1, name="psA")
                psB = pmm.tile([128, 512], F32, tag="b", bufs=1, name="psB")
                # software pipeline: iteration k emits scores(k+1), exp(k),
                # then AV(k-1) — AV trails exp by a full step so the tensor
                # engine never waits on the activation engine
                prev = None
                for k in range(mlast + 2):
                    if k <= mlast:
                        ps = psprev
                        if k < mlast:
                            psprev = issue_scores(p, j, k + 1)
                        elif bi + 1 < len(blocks):
                            # RAW safety: next block's Q/K producer copies must
                            # be emitted before any instruction reading them
                            drain_to(need_idx[blocks[bi + 1]])
                            psprev = issue_scores(*blocks[bi + 1], 0)
                        t = k - 4 * j
                        w0 = 128 * t if t > 0 else 0
                        e = work.tile([128, 2, 512], BF16, tag="e", bufs=3, name="e")
                        nc.scalar.activation(
                            e[:, :, w0:512], ps[:, :, w0:512], Exp, scale=0.125)
                        if t >= 0:
                            # partial 128-col diagonal band: zero q < k entries
                            # (gpsimd: keeps the DVE queue off the AV path)
                            nc.gpsimd.tensor_tensor(
                                out=e[:, 0, w0:w0 + 128], in0=e[:, 0, w0:w0 + 128],
                                in1=maskb[:], op=mult)
                            nc.gpsimd.tensor_tensor(
                                out=e[:, 1, w0:w0 + 128], in0=e[:, 1, w0:w0 + 128],
                                in1=maskb[:], op=mult)
                        cur = (e, k, w0)
                    else:
                        cur = None
                    if prev is not None:
                        e_, m_, w0_ = prev
                        nc.tensor.matmul(
                            out=psA[:, w0_:512],
                            lhsT=Vaug[:, p, 0, 128 * m_:128 * (m_ + 1)],
                            rhs=e_[:, 0, w0_:512],
                            start=(m_ == 0), stop=(m_ == mlast),
                        )
                        nc.tensor.matmul(
                            out=psB[:, w0_:512],
                            lhsT=Vaug[:, p, 1, 128 * m_:128 * (m_ + 1)],
                            rhs=e_[:, 1, w0_:512],
                            start=(m_ == 0), stop=(m_ == mlast),
                        )
                    prev = cur
                    if k < mlast + 1:
                        drip(rate)
                # normalize: ctxp[p][0:64|64:128, j-cols] = psX[0:64] / sums.
                # raw/sums copied out first so the psA/psB banks free up before
                # the next block's first AV matmuls need them
                sumsE = work.tile([64, 512], F32, tag="sE", bufs=2, name="sumsE")
                nc.vector.tensor_copy(out=sumsE[:], in_=psA[64:128, :])
                rawE = work.tile([64, 512], F32, tag="wE", bufs=2, name="rawE")
                nc.vector.tensor_copy(out=rawE[:], in_=psA[0:64, :])
                sumsO = work.tile([64, 512], F32, tag="sO", bufs=2, name="sumsO")
                nc.vector.tensor_copy(out=sumsO[:], in_=psB[64:128, :])
                rawO = work.tile([64, 512], F32, tag="wO", bufs=2, name="rawO")
                nc.vector.tensor_copy(out=rawO[:], in_=psB[0:64, :])
                recE = work.tile([64, 512], F32, tag="rE", bufs=2, name="recE")
                nc.vector.reciprocal_approx_fast(recE[:], sumsE[:])
                recO = work.tile([64, 512], F32, tag="rO", bufs=2, name="recO")
                nc.vector.reciprocal_approx_fast(recO[:], sumsO[:])
                nc.gpsimd.tensor_tensor(
                    out=ctxp[p][0:64, 512 * j:512 * (j + 1)],
                    in0=rawE[:], in1=recE[:], op=mult)
                codd = work.tile([64, 512], BF16, tag="cO", bufs=2, name="codd")
                nc.gpsimd.tensor_tensor(
                    out=codd[:], in0=rawO[:], in1=recO[:], op=mult)
                nc.gpsimd.tensor_copy(
                    out=ctxp[p][64:128, 512 * j:512 * (j + 1)], in_=codd[:])
                if p == 1:
                    for qm in range(4 * j, 4 * j + 4):
                        FQ += outproj_items(qm)
                drip(8)
            drain_to(len(FQ))

    nc.finalize()
    return nc


def kernel(x, qkv_w, qkv_b, out_w, out_b):
    from concourse import bass_utils
    from ml_dtypes import bfloat16
    global last_exec_time_ns

    x = np.ascontiguousarray(np.asarray(x, dtype=np.float32))
    qkv_w = np.asarray(qkv_w, dtype=np.float32)
    qkv_b = np.asarray(qkv_b, dtype=np.float32)
    out_w = np.asarray(out_w, dtype=np.float32)
    out_b = np.asarray(out_b, dtype=np.float32)

    has_qkvb = bool(np.any(qkv_b))
    if has_qkvb not in _cache:
        _cache[has_qkvb] = _build(has_qkvb)
    nc = _cache[has_qkvb]

    in_maps = []
    for c in range(N_CORES):
        b, hg = divmod(c, HG)
        xT = x[b].T.astype(bfloat16)                       # [1024, 2048]
        xTh = np.ascontiguousarray(
            xT.reshape(8, 128, S).transpose(1, 0, 2))      # [128, 8, 2048]
        rows = np.concatenate([
            qkv_w[DL * hg:DL * (hg + 1)],
            qkv_w[D_MODEL + DL * hg:D_MODEL + DL * (hg + 1)],
            qkv_w[2 * D_MODEL + DL * hg:2 * D_MODEL + DL * (hg + 1)],
        ], axis=0)
        wqT = rows.T.astype(bfloat16)                      # [1024, 768]
        # column order [Q-pair0 | K-pair0 | V | Q-pair1 | K-pair1] (see MIOFF)
        wqT = np.concatenate([
            wqT[:, 0:128], wqT[:, 256:384], wqT[:, 512:768],
            wqT[:, 128:256], wqT[:, 384:512]], axis=1)
        wqh = np.ascontiguousarray(
            wqT.reshape(8, 128, 3 * DL).transpose(1, 0, 2))
        woT = out_w[:, DL * hg:DL * (hg + 1)].T.astype(bfloat16)  # [256, 1024]
        woh = np.ascontiguousarray(
            woT.reshape(2, 128, D_MODEL).transpose(1, 0, 2))
        m = {"xTh": xTh, "wqh": wqh, "woh": woh}
        if has_qkvb:
            qb = np.concatenate([
                qkv_b[DL * hg:DL * (hg + 1)],
                qkv_b[D_MODEL + DL * hg:D_MODEL + DL * (hg + 1)],
                qkv_b[2 * D_MODEL + DL * hg:2 * D_MODEL + DL * (hg + 1)],
            ])
            qb = np.concatenate([
                qb[0:128], qb[256:384], qb[512:768], qb[128:256], qb[384:512]])
            m["qb"] = qb.reshape(1, 3 * DL).astype(bfloat16)
        in_maps.append(m)

    res = bass_utils.run_bass_kernel_spmd(nc, in_maps, core_ids=list(range(N_CORES)))
    last_exec_time_ns = res.exec_time_ns

    out = np.zeros((B, S, D_MODEL), dtype=np.float32)
    for c in range(N_CORES):
        b, hg = divmod(c, HG)
        out[b] += res.results[c]["out"].astype(np.float32)
    out += out_b[None, None, :]
    return out


# revision 23
# speedup vs baseline: 1.0160x; 1.0160x over previous
import sys

sys.path.insert(0, "/opt/trn_rl_repo")

import numpy as np

D_MODEL = 1024
NUM_HEADS = 16
HEAD_DIM = 64
B = 2
S = 2048
N_CORES = 8
HG = 4          # head-groups (cores per batch)
HPC = 4         # heads per core
DL = 256        # local feature width per core (HPC * HEAD_DIM)

_cache = {}
last_exec_time_ns = None

DRIP = 4        # filler instructions interleaved per attention m-step


def _build(has_qkvb):
    import concourse.bacc as bacc
    import concourse.mybir as mybir
    import concourse.tile as tile

    F32 = mybir.dt.float32
    F32R = mybir.dt.float32r
    BF16 = mybir.dt.bfloat16
    Exp = mybir.ActivationFunctionType.Exp
    mult = mybir.AluOpType.mult
    is_ge = mybir.AluOpType.is_ge

    nc = bacc.Bacc("TRN2", target_bir_lowering=False, debug=False)
    # packed layouts: [128 partitions, tile index, cols]
    xT_d = nc.dram_tensor("xTh", (128, 8, S), BF16, kind="ExternalInput")
    wq_d = nc.dram_tensor("wqh", (128, 8, 3 * DL), BF16, kind="ExternalInput")
    wo_d = nc.dram_tensor("woh", (128, 2, D_MODEL), BF16, kind="ExternalInput")
    if has_qkvb:
        qb_d = nc.dram_tensor("qb", (1, 3 * DL), BF16, kind="ExternalInput")
    ident_d = nc.dram_tensor("ident", (128, 128), BF16, kind="ExternalInput")
    tneg_d = nc.dram_tensor("tneg", (128, 128), BF16, kind="ExternalInput")
    out_d = nc.dram_tensor("out", (S, D_MODEL), BF16, kind="ExternalOutput")

    with tile.TileContext(nc) as tc:
        with tc.tile_pool(name="persist", bufs=1) as persist, \
             tc.tile_pool(name="work", bufs=1) as work, \
             tc.tile_pool(name="pmm", bufs=1, space="PSUM") as pmm:

            xtb = persist.tile([128, 8, S], BF16, name="xtb")
            wqb = persist.tile([128, 8, 3 * DL], BF16, name="wqb")
            wob = persist.tile([128, 2, D_MODEL], BF16, name="wob")
            # Q/K packed per head-pair p: partitions 0:64 head 2p, 64:128 head 2p+1
            QT = [persist.tile([128, S], BF16, name=f"QT{p}") for p in range(2)]
            KT = [persist.tile([128, S], BF16, name=f"KT{p}") for p in range(2)]
            # V augmented [pair, parity, S]: per key-tile block of 128 cols:
            # [V dims 64 | ones 64]; partitions of block st = keys of tile st
            Vaug = persist.tile([128, 2, 2, S], BF16, name="Vaug")
            ctxp = [persist.tile([128, S], BF16, name=f"ctxp{p}") for p in range(2)]
            identb = persist.tile([128, 128], BF16, name="identb")
            tnegb = persist.tile([128, 128], BF16, name="tnegb")

            # DMAs in need order. wq columns are host-reordered as
            # [Q-pair0 | K-pair0 | V | Q-pair1 | K-pair1] so the critical
            # pair-0 QK weights land first. The first chunk is split per
            # i-tile so the leading projection matmuls start as soon as
            # their own slice arrives.
            nc.sync.dma_start(out=wqb[:, :, 0:256], in_=wq_d[:, :, 0:256])
            for i in range(8):
                nc.sync.dma_start(out=xtb[:, i, 0:512], in_=xT_d[:, i, 0:512])
            nc.sync.dma_start(out=wqb[:, :, 256:512], in_=wq_d[:, :, 256:512])
            if has_qkvb:
                qb_t = persist.tile([1, 3 * DL], BF16, name="qb_t")
                nc.sync.dma_start(out=qb_t[:], in_=qb_d[:])
                ones_t = persist.tile([1, 512], BF16, name="ones_t")
                nc.vector.memset(ones_t[:], 1.0)
            nc.sync.dma_start(out=wob[:], in_=wo_d[:])
            nc.sync.dma_start(out=xtb[:, :, 512:1024], in_=xT_d[:, :, 512:1024])
            nc.sync.dma_start(out=wqb[:, :, 512:768], in_=wq_d[:, :, 512:768])
            nc.sync.dma_start(out=xtb[:, :, 1024:1536], in_=xT_d[:, :, 1024:1536])
            nc.sync.dma_start(out=xtb[:, :, 1536:2048], in_=xT_d[:, :, 1536:2048])

            # causal masking is folded into the scores matmul: the band tile
            # accumulates ident.T @ tneg, where tneg[k, c] = -240 for c < k.
            # exp(0.125 * (s - 240)) ~ 3e-14 zeroes the dead region. Both
            # constants come from the host.
            nc.sync.dma_start(out=identb[:], in_=ident_d[:])
            nc.sync.dma_start(out=tnegb[:], in_=tneg_d[:])
            nc.vector.memset(Vaug[:], 1.0)

            # ---- projection / out-projection instruction generators ----
            MIOFF = {0: 0, 2: 128, 1: 512, 3: 640}

            def qk_items(mi, n):
                # psq = sum_i wq[i][:, mi-block].T @ xt[i][:, n-chunk]  -> [128 feat, 512 seq]
                items = []
                st = {}

                def mk(i):
                    def f():
                        if i == 0:
                            st['ps'] = pmm.tile([128, 512], F32, tag="pp", bufs=2, name="psq")
                        nc.tensor.matmul(
                            out=st['ps'][:],
                            lhsT=wqb[:, i, MIOFF[mi]:MIOFF[mi] + 128],
                            rhs=xtb[:, i, 512 * n:512 * (n + 1)],
                            start=(i == 0),
                            stop=(i == 7 and not has_qkvb),
                        )
                    return f
                for i in range(8):
                    items.append(mk(i))
                if has_qkvb:
                    def fb():
                        nc.tensor.matmul(
                            out=st['ps'][:],
                            lhsT=qb_t[0:1, MIOFF[mi]:MIOFF[mi] + 128],
                            rhs=ones_t[0:1, :],
                            start=False, stop=True,
                        )
                    items.append(fb)

                def cp():
                    dst = QT[mi] if mi < 2 else KT[mi - 2]
                    nc.vector.tensor_copy(
                        out=dst[:, 512 * n:512 * (n + 1)], in_=st['ps'][:])
                items.append(cp)
                return items

            def v_items(sti):
                # psv = sum_i xt[i][:, st-block].T @ wq[i][:, V cols] -> [128 seq, 256 feat]
                items = []
                st = {}

                def mk(i):
                    def f():
                        if i == 0:
                            st['ps'] = pmm.tile([128, 2, 128], F32, tag="pp", bufs=2, name="psv")
                        nc.tensor.matmul(
                            out=st['ps'][:],
                            lhsT=xtb[:, i, 128 * sti:128 * (sti + 1)],
                            rhs=wqb[:, i, 256:512],
                            start=(i == 0),
                            stop=(i == 7 and not has_qkvb),
                        )
                    return f
                for i in range(8):
                    items.append(mk(i))
                if has_qkvb:
                    def fb():
                        nc.tensor.matmul(
                            out=st['ps'][:],
                            lhsT=ones_t[0:1, 0:128],
                            rhs=qb_t[0:1, 256:512],
                            start=False, stop=True,
                        )
                    items.append(fb)

                def cpe():
                    nc.vector.tensor_copy(
                        out=Vaug[:, :, 0, 128 * sti:128 * sti + 64],
                        in_=st['ps'][:, :, 0:64])

                def cpo():
                    nc.vector.tensor_copy(
                        out=Vaug[:, :, 1, 128 * sti:128 * sti + 64],
                        in_=st['ps'][:, :, 64:128])
                items.append(cpe)
                items.append(cpo)
                return items

            def outproj_items(qm):
                items = []
                st = {}

                def half(n):
                    def f():
                        ps = pmm.tile([128, 512], F32, tag="pp", bufs=2, name="pso")
                        nc.tensor.matmul(
                            out=ps[:],
                            lhsT=ctxp[0][:, 128 * qm:128 * (qm + 1)],
                            rhs=wob[:, 0, 512 * n:512 * (n + 1)],
                            start=True, stop=False,
                        )
                        nc.tensor.matmul(
                            out=ps[:],
                            lhsT=ctxp[1][:, 128 * qm:128 * (qm + 1)],
                            rhs=wob[:, 1, 512 * n:512 * (n + 1)],
                            start=False, stop=True,
                        )
                        if n == 0:
                            st['stage'] = work.tile([128, D_MODEL], BF16, tag="st", bufs=3, name="stage")
                        nc.vector.tensor_copy(
                            out=st['stage'][:, 512 * n:512 * (n + 1)], in_=ps[:])
                    return f
                items.append(half(0))
                items.append(half(1))

                def dm():
                    nc.sync.dma_start(out=out_d[128 * qm:128 * (qm + 1), :], in_=st['stage'][:])
                items.append(dm)
                return items

            # ---- attention scores issue ----
            def issue_scores(p, j, m):
                t = m - 4 * j
                lo = 128 * t if t > 0 else 0
                band = t >= 0
                ps = pmm.tile([128, 2, 512], F32, tag="s", bufs=2, name="psS")
                for h, pr in ((0, slice(0, 64)), (1, slice(64, 128))):
                    nc.tensor.matmul(
                        out=ps[:, h, lo:512],
                        lhsT=KT[p][pr, 128 * m:128 * (m + 1)],
                        rhs=QT[p][pr, 512 * j + lo:512 * (j + 1)],
                        start=True, stop=not band,
                    )
                    if band:
                        nc.tensor.matmul(
                            out=ps[:, h, lo:lo + 128],
                            lhsT=identb[:],
                            rhs=tnegb[:],
                            start=False, stop=True,
                        )
                return ps

            # ---- immediate emission: minimum needed for attn(0, 0) ----
            # qk(0,0) and qk(2,0) interleaved per i so each matmul runs as its
            # x/w slice lands
            psq0 = pmm.tile([128, 512], F32, tag="pp", bufs=2, name="psq0")
            psq2 = pmm.tile([128, 512], F32, tag="pp", bufs=2, name="psq2")
            for i in range(8):
                for mi, pst in ((0, psq0), (2, psq2)):
                    nc.tensor.matmul(
                        out=pst[:],
                        lhsT=wqb[:, i, MIOFF[mi]:MIOFF[mi] + 128],
                        rhs=xtb[:, i, 0:512],
                        start=(i == 0),
                        stop=(i == 7 and not has_qkvb),
                    )
            if has_qkvb:
                for mi, pst in ((0, psq0), (2, psq2)):
                    nc.tensor.matmul(
                        out=pst[:],
                        lhsT=qb_t[0:1, MIOFF[mi]:MIOFF[mi] + 128],
                        rhs=ones_t[0:1, :],
                        start=False, stop=True,
                    )
            nc.vector.tensor_copy(out=QT[0][:, 0:512], in_=psq0[:])
            nc.vector.tensor_copy(out=KT[0][:, 0:512], in_=psq2[:])
            psprev = issue_scores(0, 0, 0)
            for sti in range(4):
                for it in v_items(sti):
                    it()

            # ---- filler queue for the rest, drained during attention ----
            FQ = []
            need_idx = {(0, 0): 0}
            for n in range(1, 4):
                FQ += qk_items(0, n) + qk_items(2, n)
                for sti in range(4 * n, 4 * n + 4):
                    FQ += v_items(sti)
                need_idx[(0, n)] = len(FQ)
            for n in range(4):
                FQ += qk_items(1, n) + qk_items(3, n)
                need_idx[(1, n)] = len(FQ)

            drained = [0]

            def drain_to(k):
                while drained[0] < k:
                    FQ[drained[0]]()
                    drained[0] += 1

            def drip(r):
                drain_to(min(drained[0] + r, len(FQ)))

            # pair-1 blocks in descending j so the final out-proj chunk is small
            blocks = [(0, 0), (0, 1), (0, 2), (0, 3), (1, 3), (1, 2), (1, 1), (1, 0)]
            # late blocks drip slower so out-proj filler spreads across the
            # starved final phase instead of draining in a burst
            drip_rate = {(1, 3): 4, (1, 2): 2, (1, 1): 2, (1, 0): 3}
            for bi, (p, j) in enumerate(blocks):
                drain_to(need_idx[(p, j)])
                rate = drip_rate.get((p, j), DRIP)
                mlast = 4 * j + 3
                psA = pmm.tile([128, 512], F32, tag="a", bufs=1, name="psA")
                psB = pmm.tile([128, 512], F32, tag="b", bufs=1, name="psB")
                # software pipeline: iteration k emits scores(k+1), exp(k),
                # then AV(k-1) — AV trails exp by a full step so the tensor
                # engine never waits on the activation engine
                prev = None
                for k in range(mlast + 2):
                    if k <= mlast:
                        ps = psprev
                        if k < mlast:
                            psprev = issue_scores(p, j, k + 1)
                        elif bi + 1 < len(blocks):
                            # RAW safety: next block's Q/K producer copies must
                            # be emitted before any instruction reading them
                            drain_to(need_idx[blocks[bi + 1]])
                            psprev = issue_scores(*blocks[bi + 1], 0)
                        t = k - 4 * j
                        w0 = 128 * t if t > 0 else 0
                        e = work.tile([128, 2, 512], BF16, tag="e", bufs=3, name="e")
                        nc.scalar.activation(
                            e[:, :, w0:512], ps[:, :, w0:512], Exp, scale=0.125)
                        cur = (e, k, w0)
                    else:
                        cur = None
                    if prev is not None:
                        e_, m_, w0_ = prev
                        nc.tensor.matmul(
                            out=psA[:, w0_:512],
                            lhsT=Vaug[:, p, 0, 128 * m_:128 * (m_ + 1)],
                            rhs=e_[:, 0, w0_:512],
                            start=(m_ == 0), stop=(m_ == mlast),
                        )
                        nc.tensor.matmul(
                            out=psB[:, w0_:512],
                            lhsT=Vaug[:, p, 1, 128 * m_:128 * (m_ + 1)],
                            rhs=e_[:, 1, w0_:512],
                            start=(m_ == 0), stop=(m_ == mlast),
                        )
                    prev = cur
                    if k < mlast + 1:
                        drip(rate)
                # normalize: ctxp[p][0:64|64:128, j-cols] = psX[0:64] / sums.
                # raw/sums copied out first so the psA/psB banks free up before
                # the next block's first AV matmuls need them
                sumsE = work.tile([64, 512], F32, tag="sE", bufs=2, name="sumsE")
                nc.vector.tensor_copy(out=sumsE[:], in_=psA[64:128, :])
                rawE = work.tile([64, 512], F32, tag="wE", bufs=2, name="rawE")
                nc.vector.tensor_copy(out=rawE[:], in_=psA[0:64, :])
                sumsO = work.tile([64, 512], F32, tag="sO", bufs=2, name="sumsO")
                nc.vector.tensor_copy(out=sumsO[:], in_=psB[64:128, :])
                rawO = work.tile([64, 512], F32, tag="wO", bufs=2, name="rawO")
                nc.vector.tensor_copy(out=rawO[:], in_=psB[0:64, :])
                recE = work.tile([64, 512], F32, tag="rE", bufs=2, name="recE")
                nc.vector.reciprocal_approx_fast(recE[:], sumsE[:])
                nc.vector.tensor_tensor(
                    out=ctxp[p][0:64, 512 * j:512 * (j + 1)],
                    in0=rawE[:], in1=recE[:], op=mult)
                recO = work.tile([64, 512], F32, tag="rO", bufs=2, name="recO")
                nc.vector.reciprocal_approx_fast(recO[:], sumsO[:])
                codd = work.tile([64, 512], BF16, tag="cO", bufs=2, name="codd")
                nc.vector.tensor_tensor(
                    out=codd[:], in0=rawO[:], in1=recO[:], op=mult)
                nc.vector.tensor_copy(
                    out=ctxp[p][64:128, 512 * j:512 * (j + 1)], in_=codd[:])
                if p == 1:
                    for qm in range(4 * j, 4 * j + 4):
                        FQ += outproj_items(qm)
                drip(8)
            drain_to(len(FQ))

    nc.finalize()
    return nc


def _ident():
    from ml_dtypes import bfloat16
    return np.eye(128, dtype=np.float32).astype(bfloat16)


def _tneg():
    from ml_dtypes import bfloat16
    k = np.arange(128)[:, None]
    c = np.arange(128)[None, :]
    return np.where(c < k, np.float32(-240.0), np.float32(0.0)).astype(bfloat16)


def kernel(x, qkv_w, qkv_b, out_w, out_b):
    from concourse import bass_utils
    from ml_dtypes import bfloat16
    global last_exec_time_ns

    x = np.ascontiguousarray(np.asarray(x, dtype=np.float32))
    qkv_w = np.asarray(qkv_w, dtype=np.float32)
    qkv_b = np.asarray(qkv_b, dtype=np.float32)
    out_w = np.asarray(out_w, dtype=np.float32)
    out_b = np.asarray(out_b, dtype=np.float32)

    has_qkvb = bool(np.any(qkv_b))
    if has_qkvb not in _cache:
        _cache[has_qkvb] = _build(has_qkvb)
    nc = _cache[has_qkvb]

    in_maps = []
    for c in range(N_CORES):
        b, hg = divmod(c, HG)
        xT = x[b].T.astype(bfloat16)                       # [1024, 2048]
        xTh = np.ascontiguousarray(
            xT.reshape(8, 128, S).transpose(1, 0, 2))      # [128, 8, 2048]
        rows = np.concatenate([
            qkv_w[DL * hg:DL * (hg + 1)],
            qkv_w[D_MODEL + DL * hg:D_MODEL + DL * (hg + 1)],
            qkv_w[2 * D_MODEL + DL * hg:2 * D_MODEL + DL * (hg + 1)],
        ], axis=0)
        wqT = rows.T.astype(bfloat16)                      # [1024, 768]
        # column order [Q-pair0 | K-pair0 | V | Q-pair1 | K-pair1] (see MIOFF)
        wqT = np.concatenate([
            wqT[:, 0:128], wqT[:, 256:384], wqT[:, 512:768],
            wqT[:, 128:256], wqT[:, 384:512]], axis=1)
        wqh = np.ascontiguousarray(
            wqT.reshape(8, 128, 3 * DL).transpose(1, 0, 2))
        woT = out_w[:, DL * hg:DL * (hg + 1)].T.astype(bfloat16)  # [256, 1024]
        woh = np.ascontiguousarray(
            woT.reshape(2, 128, D_MODEL).transpose(1, 0, 2))
        m = {"xTh": xTh, "wqh": wqh, "woh": woh,
             "ident": _ident(), "tneg": _tneg()}
        if has_qkvb:
            qb = np.concatenate([
                qkv_b[DL * hg:DL * (hg + 1)],
                qkv_b[D_MODEL + DL * hg:D_MODEL + DL * (hg + 1)],
                qkv_b[2 * D_MODEL + DL * hg:2 * D_MODEL + DL * (hg + 1)],
            ])
            qb = np.concatenate([
                qb[0:128], qb[256:384], qb[512:768], qb[128:256], qb[384:512]])
            m["qb"] = qb.reshape(1, 3 * DL).astype(bfloat16)
        in_maps.append(m)

    res = bass_utils.run_bass_kernel_spmd(nc, in_maps, core_ids=list(range(N_CORES)))
    last_exec_time_ns = res.exec_time_ns

    out = np.zeros((B, S, D_MODEL), dtype=np.float32)
    for c in range(N_CORES):
        b, hg = divmod(c, HG)
        out[b] += res.results[c]["out"].astype(np.float32)
    out += out_b[None, None, :]
    return out


# revision 24
# speedup vs baseline: 1.0471x; 1.0305x over previous
import sys

sys.path.insert(0, "/opt/trn_rl_repo")

import numpy as np

D_MODEL = 1024
NUM_HEADS = 16
HEAD_DIM = 64
B = 2
S = 2048
N_CORES = 8
HG = 4          # head-groups (cores per batch)
HPC = 4         # heads per core
DL = 256        # local feature width per core (HPC * HEAD_DIM)

_cache = {}
last_exec_time_ns = None

DRIP = 4        # filler instructions interleaved per attention m-step


def _build(has_qkvb):
    import concourse.bacc as bacc
    import concourse.mybir as mybir
    import concourse.tile as tile

    F32 = mybir.dt.float32
    F32R = mybir.dt.float32r
    BF16 = mybir.dt.bfloat16
    Exp = mybir.ActivationFunctionType.Exp
    mult = mybir.AluOpType.mult
    is_ge = mybir.AluOpType.is_ge

    nc = bacc.Bacc("TRN2", target_bir_lowering=False, debug=False)
    # packed layouts: [128 partitions, tile index, cols]
    xT_d = nc.dram_tensor("xTh", (128, 8, S), BF16, kind="ExternalInput")
    wq_d = nc.dram_tensor("wqh", (128, 8, 3 * DL), BF16, kind="ExternalInput")
    wo_d = nc.dram_tensor("woh", (128, 2, D_MODEL), BF16, kind="ExternalInput")
    if has_qkvb:
        qb_d = nc.dram_tensor("qb", (1, 3 * DL), BF16, kind="ExternalInput")
    ident_d = nc.dram_tensor("ident", (128, 128), BF16, kind="ExternalInput")
    tneg_d = nc.dram_tensor("tneg", (128, 128), BF16, kind="ExternalInput")
    out_d = nc.dram_tensor("out", (S, D_MODEL), BF16, kind="ExternalOutput")

    with tile.TileContext(nc) as tc:
        with tc.tile_pool(name="persist", bufs=1) as persist, \
             tc.tile_pool(name="work", bufs=1) as work, \
             tc.tile_pool(name="pmm", bufs=1, space="PSUM") as pmm:

            xtb = persist.tile([128, 8, S], BF16, name="xtb")
            wqb = persist.tile([128, 8, 3 * DL], BF16, name="wqb")
            wob = persist.tile([128, 2, D_MODEL], BF16, name="wob")
            # Q/K packed per head-pair p: partitions 0:64 head 2p, 64:128 head 2p+1
            QT = [persist.tile([128, S], BF16, name=f"QT{p}") for p in range(2)]
            KT = [persist.tile([128, S], BF16, name=f"KT{p}") for p in range(2)]
            # V augmented [pair, parity, S]: per key-tile block of 128 cols:
            # [V dims 64 | ones 64]; partitions of block st = keys of tile st
            Vaug = persist.tile([128, 2, 2, S], BF16, name="Vaug")
            ctxp = [persist.tile([128, S], BF16, name=f"ctxp{p}") for p in range(2)]
            identb = persist.tile([128, 128], BF16, name="identb")
            tnegb = persist.tile([128, 128], BF16, name="tnegb")

            # DMAs in need order. wq columns are host-reordered as
            # [Q-pair0 | K-pair0 | V | Q-pair1 | K-pair1] so the critical
            # pair-0 QK weights land first. The first chunk is split per
            # i-tile so the leading projection matmuls start as soon as
            # their own slice arrives.
            nc.sync.dma_start(out=wqb[:, :, 0:256], in_=wq_d[:, :, 0:256])
            for i in range(8):
                nc.sync.dma_start(out=xtb[:, i, 0:512], in_=xT_d[:, i, 0:512])
            nc.sync.dma_start(out=wqb[:, :, 256:512], in_=wq_d[:, :, 256:512])
            if has_qkvb:
                qb_t = persist.tile([1, 3 * DL], BF16, name="qb_t")
                nc.sync.dma_start(out=qb_t[:], in_=qb_d[:])
                ones_t = persist.tile([1, 512], BF16, name="ones_t")
                nc.vector.memset(ones_t[:], 1.0)
            nc.sync.dma_start(out=wob[:], in_=wo_d[:])
            nc.sync.dma_start(out=xtb[:, :, 512:1024], in_=xT_d[:, :, 512:1024])
            nc.sync.dma_start(out=wqb[:, :, 512:768], in_=wq_d[:, :, 512:768])
            nc.sync.dma_start(out=xtb[:, :, 1024:1536], in_=xT_d[:, :, 1024:1536])
            nc.sync.dma_start(out=xtb[:, :, 1536:2048], in_=xT_d[:, :, 1536:2048])

            # causal masking is folded into the scores matmul: the band tile
            # accumulates ident.T @ tneg, where tneg[k, c] = -240 for c < k.
            # exp(0.125 * (s - 240)) ~ 3e-14 zeroes the dead region. Both
            # constants come from the host.
            nc.sync.dma_start(out=identb[:], in_=ident_d[:])
            nc.sync.dma_start(out=tnegb[:], in_=tneg_d[:])
            nc.vector.memset(Vaug[:], 1.0)

            # ---- projection / out-projection instruction generators ----
            MIOFF = {0: 0, 2: 128, 1: 512, 3: 640}

            def qk_items(mi, n):
                # psq = sum_i wq[i][:, mi-block].T @ xt[i][:, n-chunk]  -> [128 feat, 512 seq]
                items = []
                st = {}

                def mk(i):
                    def f():
                        if i == 0:
                            st['ps'] = pmm.tile([128, 512], F32, tag="pp", bufs=2, name="psq")
                        nc.tensor.matmul(
                            out=st['ps'][:],
                            lhsT=wqb[:, i, MIOFF[mi]:MIOFF[mi] + 128],
                            rhs=xtb[:, i, 512 * n:512 * (n + 1)],
                            start=(i == 0),
                            stop=(i == 7 and not has_qkvb),
                        )
                    return f
                for i in range(8):
                    items.append(mk(i))
                if has_qkvb:
                    def fb():
                        nc.tensor.matmul(
                            out=st['ps'][:],
                            lhsT=qb_t[0:1, MIOFF[mi]:MIOFF[mi] + 128],
                            rhs=ones_t[0:1, :],
                            start=False, stop=True,
                        )
                    items.append(fb)

                def cp():
                    dst = QT[mi] if mi < 2 else KT[mi - 2]
                    nc.vector.tensor_copy(
                        out=dst[:, 512 * n:512 * (n + 1)], in_=st['ps'][:])
                items.append(cp)
                return items

            def v_items(sti):
                # psv = sum_i xt[i][:, st-block].T @ wq[i][:, V cols] -> [128 seq, 256 feat]
                items = []
                st = {}

                def mk(i):
                    def f():
                        if i == 0:
                            st['ps'] = pmm.tile([128, 2, 128], F32, tag="pp", bufs=2, name="psv")
                        nc.tensor.matmul(
                            out=st['ps'][:],
                            lhsT=xtb[:, i, 128 * sti:128 * (sti + 1)],
                            rhs=wqb[:, i, 256:512],
                            start=(i == 0),
                            stop=(i == 7 and not has_qkvb),
                        )
                    return f
                for i in range(8):
                    items.append(mk(i))
                if has_qkvb:
                    def fb():
                        nc.tensor.matmul(
                            out=st['ps'][:],
                            lhsT=ones_t[0:1, 0:128],
                            rhs=qb_t[0:1, 256:512],
                            start=False, stop=True,
                        )
                    items.append(fb)

                def cpe():
                    nc.vector.tensor_copy(
                        out=Vaug[:, :, 0, 128 * sti:128 * sti + 64],
                        in_=st['ps'][:, :, 0:64])

                def cpo():
                    nc.vector.tensor_copy(
                        out=Vaug[:, :, 1, 128 * sti:128 * sti + 64],
                        in_=st['ps'][:, :, 64:128])
                items.append(cpe)
                items.append(cpo)
                return items

            def outproj_items(qm, act_half=False):
                items = []
                st = {}

                def half(n):
                    def f():
                        ps = pmm.tile([128, 512], F32, tag="pp", bufs=2, name="pso")
                        nc.tensor.matmul(
                            out=ps[:],
                            lhsT=ctxp[0][:, 128 * qm:128 * (qm + 1)],
                            rhs=wob[:, 0, 512 * n:512 * (n + 1)],
                            start=True, stop=False,
                        )
                        nc.tensor.matmul(
                            out=ps[:],
                            lhsT=ctxp[1][:, 128 * qm:128 * (qm + 1)],
                            rhs=wob[:, 1, 512 * n:512 * (n + 1)],
                            start=False, stop=True,
                        )
                        if n == 0:
                            st['stage'] = work.tile([128, D_MODEL], BF16, tag="st", bufs=3, name="stage")
                        if n == 1 and act_half:
                            nc.scalar.copy(
                                out=st['stage'][:, 512:1024], in_=ps[:])
                        else:
                            nc.vector.tensor_copy(
                                out=st['stage'][:, 512 * n:512 * (n + 1)], in_=ps[:])
                    return f
                items.append(half(0))
                items.append(half(1))

                def dm():
                    nc.sync.dma_start(out=out_d[128 * qm:128 * (qm + 1), :], in_=st['stage'][:])
                items.append(dm)
                return items

            # ---- attention scores issue ----
            def issue_scores(p, j, m):
                t = m - 4 * j
                lo = 128 * t if t > 0 else 0
                band = t >= 0
                ps = pmm.tile([128, 2, 512], F32, tag="s", bufs=2, name="psS")
                for h, pr in ((0, slice(0, 64)), (1, slice(64, 128))):
                    nc.tensor.matmul(
                        out=ps[:, h, lo:512],
                        lhsT=KT[p][pr, 128 * m:128 * (m + 1)],
                        rhs=QT[p][pr, 512 * j + lo:512 * (j + 1)],
                        start=True, stop=not band,
                    )
                    if band:
                        nc.tensor.matmul(
                            out=ps[:, h, lo:lo + 128],
                            lhsT=identb[:],
                            rhs=tnegb[:],
                            start=False, stop=True,
                        )
                return ps

            # ---- immediate emission: minimum needed for attn(0, 0) ----
            # qk(0,0) and qk(2,0) interleaved per i so each matmul runs as its
            # x/w slice lands
            psq0 = pmm.tile([128, 512], F32, tag="pp", bufs=2, name="psq0")
            psq2 = pmm.tile([128, 512], F32, tag="pp", bufs=2, name="psq2")
            for i in range(8):
                for mi, pst in ((0, psq0), (2, psq2)):
                    nc.tensor.matmul(
                        out=pst[:],
                        lhsT=wqb[:, i, MIOFF[mi]:MIOFF[mi] + 128],
                        rhs=xtb[:, i, 0:512],
                        start=(i == 0),
                        stop=(i == 7 and not has_qkvb),
                    )
            if has_qkvb:
                for mi, pst in ((0, psq0), (2, psq2)):
                    nc.tensor.matmul(
                        out=pst[:],
                        lhsT=qb_t[0:1, MIOFF[mi]:MIOFF[mi] + 128],
                        rhs=ones_t[0:1, :],
                        start=False, stop=True,
                    )
            nc.vector.tensor_copy(out=QT[0][:, 0:512], in_=psq0[:])
            nc.vector.tensor_copy(out=KT[0][:, 0:512], in_=psq2[:])
            psprev = issue_scores(0, 0, 0)
            for sti in range(4):
                for it in v_items(sti):
                    it()

            # ---- filler queue for the rest, drained during attention ----
            FQ = []
            need_idx = {(0, 0): 0}
            for n in range(1, 4):
                FQ += qk_items(0, n) + qk_items(2, n)
                for sti in range(4 * n, 4 * n + 4):
                    FQ += v_items(sti)
                need_idx[(0, n)] = len(FQ)
            for n in range(4):
                FQ += qk_items(1, n) + qk_items(3, n)
                need_idx[(1, n)] = len(FQ)

            drained = [0]

            def drain_to(k):
                while drained[0] < k:
                    FQ[drained[0]]()
                    drained[0] += 1

            def drip(r):
                drain_to(min(drained[0] + r, len(FQ)))

            # pair-1 blocks in descending j so the final out-proj chunk is small
            blocks = [(0, 0), (0, 1), (0, 2), (0, 3), (1, 3), (1, 2), (1, 1), (1, 0)]
            # late blocks drip slower so out-proj filler spreads across the
            # starved final phase instead of draining in a burst
            drip_rate = {(1, 3): 4, (1, 2): 2, (1, 1): 2, (1, 0): 3}
            for bi, (p, j) in enumerate(blocks):
                drain_to(need_idx[(p, j)])
                rate = drip_rate.get((p, j), DRIP)
                mlast = 4 * j + 3
                psA = pmm.tile([128, 512], F32, tag="a", bufs=1, name="psA")
                psB = pmm.tile([128, 512], F32, tag="b", bufs=1, name="psB")
                # software pipeline: iteration k emits scores(k+1), exp(k),
                # then AV(k-1) — AV trails exp by a full step so the tensor
                # engine never waits on the activation engine
                prev = None
                for k in range(mlast + 2):
                    if k <= mlast:
                        ps = psprev
                        if k < mlast:
                            psprev = issue_scores(p, j, k + 1)
                        elif bi + 1 < len(blocks):
                            # RAW safety: next block's Q/K producer copies must
                            # be emitted before any instruction reading them
                            drain_to(need_idx[blocks[bi + 1]])
                            psprev = issue_scores(*blocks[bi + 1], 0)
                        t = k - 4 * j
                        w0 = 128 * t if t > 0 else 0
                        e = work.tile([128, 2, 512], BF16, tag="e", bufs=3, name="e")
                        nc.scalar.activation(
                            e[:, :, w0:512], ps[:, :, w0:512], Exp, scale=0.125)
                        cur = (e, k, w0)
                    else:
                        cur = None
                    if prev is not None:
                        e_, m_, w0_ = prev
                        nc.tensor.matmul(
                            out=psA[:, w0_:512],
                            lhsT=Vaug[:, p, 0, 128 * m_:128 * (m_ + 1)],
                            rhs=e_[:, 0, w0_:512],
                            start=(m_ == 0), stop=(m_ == mlast),
                        )
                        nc.tensor.matmul(
                            out=psB[:, w0_:512],
                            lhsT=Vaug[:, p, 1, 128 * m_:128 * (m_ + 1)],
                            rhs=e_[:, 1, w0_:512],
                            start=(m_ == 0), stop=(m_ == mlast),
                        )
                    prev = cur
                    if k < mlast + 1:
                        drip(rate)
                # normalize: ctxp[p][0:64|64:128, j-cols] = psX[0:64] / sums
                sumsE = work.tile([64, 512], F32, tag="sE", bufs=2, name="sumsE")
                nc.vector.tensor_copy(out=sumsE[:], in_=psA[64:128, :])
                recE = work.tile([64, 512], F32, tag="rE", bufs=2, name="recE")
                nc.vector.reciprocal_approx_fast(recE[:], sumsE[:])
                nc.vector.tensor_tensor(
                    out=ctxp[p][0:64, 512 * j:512 * (j + 1)],
                    in0=psA[0:64, :], in1=recE[:], op=mult)
                sumsO = work.tile([64, 512], F32, tag="sO", bufs=2, name="sumsO")
                nc.vector.tensor_copy(out=sumsO[:], in_=psB[64:128, :])
                recO = work.tile([64, 512], F32, tag="rO", bufs=2, name="recO")
                nc.vector.reciprocal_approx_fast(recO[:], sumsO[:])
                codd = work.tile([64, 512], BF16, tag="cO", bufs=2, name="codd")
                nc.vector.tensor_tensor(
                    out=codd[:], in0=psB[0:64, :], in1=recO[:], op=mult)
                nc.vector.tensor_copy(
                    out=ctxp[p][64:128, 512 * j:512 * (j + 1)], in_=codd[:])
                if p == 1:
                    for qm in range(4 * j, 4 * j + 4):
                        FQ += outproj_items(qm, act_half=(j <= 1))
                drip(8)
            drain_to(len(FQ))

    nc.finalize()
    return nc


def _ident():
    from ml_dtypes import bfloat16
    return np.eye(128, dtype=np.float32).astype(bfloat16)


def _tneg():
    from ml_dtypes import bfloat16
    k = np.arange(128)[:, None]
    c = np.arange(128)[None, :]
    return np.where(c < k, np.float32(-240.0), np.float32(0.0)).astype(bfloat16)


def kernel(x, qkv_w, qkv_b, out_w, out_b):
    from concourse import bass_utils
    from ml_dtypes import bfloat16
    global last_exec_time_ns

    x = np.ascontiguousarray(np.asarray(x, dtype=np.float32))
    qkv_w = np.asarray(qkv_w, dtype=np.float32)
    qkv_b = np.asarray(qkv_b, dtype=np.float32)
    out_w = np.asarray(out_w, dtype=np.float32)
    out_b = np.asarray(out_b, dtype=np.float32)

    has_qkvb = bool(np.any(qkv_b))
    if has_qkvb not in _cache:
        _cache[has_qkvb] = _build(has_qkvb)
    nc = _cache[has_qkvb]

    in_maps = []
    for c in range(N_CORES):
        b, hg = divmod(c, HG)
        xT = x[b].T.astype(bfloat16)                       # [1024, 2048]
        xTh = np.ascontiguousarray(
            xT.reshape(8, 128, S).transpose(1, 0, 2))      # [128, 8, 2048]
        rows = np.concatenate([
            qkv_w[DL * hg:DL * (hg + 1)],
            qkv_w[D_MODEL + DL * hg:D_MODEL + DL * (hg + 1)],
            qkv_w[2 * D_MODEL + DL * hg:2 * D_MODEL + DL * (hg + 1)],
        ], axis=0)
        wqT = rows.T.astype(bfloat16)                      # [1024, 768]
        # column order [Q-pair0 | K-pair0 | V | Q-pair1 | K-pair1] (see MIOFF)
        wqT = np.concatenate([
            wqT[:, 0:128], wqT[:, 256:384], wqT[:, 512:768],
            wqT[:, 128:256], wqT[:, 384:512]], axis=1)
        wqh = np.ascontiguousarray(
            wqT.reshape(8, 128, 3 * DL).transpose(1, 0, 2))
        woT = out_w[:, DL * hg:DL * (hg + 1)].T.astype(bfloat16)  # [256, 1024]
        woh = np.ascontiguousarray(
            woT.reshape(2, 128, D_MODEL).transpose(1, 0, 2))
        m = {"xTh": xTh, "wqh": wqh, "woh": woh,
             "ident": _ident(), "tneg": _tneg()}
        if has_qkvb:
            qb = np.concatenate([
                qkv_b[DL * hg:DL * (hg + 1)],
                qkv_b[D_MODEL + DL * hg:D_MODEL + DL * (hg + 1)],
                qkv_b[2 * D_MODEL + DL * hg:2 * D_MODEL + DL * (hg + 1)],
            ])
            qb = np.concatenate([
                qb[0:128], qb[256:384], qb[512:768], qb[128:256], qb[384:512]])
            m["qb"] = qb.reshape(1, 3 * DL).astype(bfloat16)
        in_maps.append(m)

    res = bass_utils.run_bass_kernel_spmd(nc, in_maps, core_ids=list(range(N_CORES)))
    last_exec_time_ns = res.exec_time_ns

    out = np.zeros((B, S, D_MODEL), dtype=np.float32)
    for c in range(N_CORES):
        b, hg = divmod(c, HG)
        out[b] += res.results[c]["out"].astype(np.float32)
    out += out_b[None, None, :]
    return out
